# revision 11
# baseline (speedup 1.0000x reference)
"""Trainium2 Bass kernel for a 2-layer Clifford-algebra (Cl(3,0)) equivariant MLP.

Contract: kernel(**inputs) takes the FULL unsharded inputs (numpy arrays, keyed
as in the problem's setup_inputs()) and returns the FULL [8192, 256, 8] float32
output.  Internally the batch dim is sharded across 8 NeuronCores (pure data
parallel; all parameters replicated), one fused Bass/Tile program per core.

Per-core algorithm (channel-major layout [c_block(2) x 128, blade(8), b]):
  layer(x):
    h0   = per-grade matmul(x, w_lin) + bias            (TensorE, bf16)
    h    = sigmoid(a*inv(h0)+b)[grade] * h0             (ACT sigmoid, DVE mult)
    yr   = per-grade matmul(h, w_right)                 (TensorE)
    xr   = yr / (sigmoid(na)*(|yr|_g - 1) + 1 + eps)    (ACT/DVE)
    left = per-grade matmul(h, w_left) + bias           (TensorE -> PSUM)
    gp   = sum_j cayley-weighted products h_i * xr_j    (DVE STT products,
                                                         TensorE identity-matmul
                                                         accumulation into the
                                                         same PSUM as `left`)
    out  = ln_a * (left+gp)/sqrt(2) / (mean_c |z| + eps) (PE column-sum + DVE)
  ln_a of layer 0 and the 1/sqrt(2) factors are folded into weights on host.
"""

import numpy as np
import ml_dtypes

BF16 = ml_dtypes.bfloat16

# ---------------------------------------------------------------- problem dims
B_FULL = 8192
C = 256
NB = 8
N_CORES = 8
BC = B_FULL // N_CORES          # batch per core
BT = 256                        # chunk of batch processed per pipeline step
NCHUNK = BC // BT
P = 128                         # partitions
CB = C // P                     # channel blocks
EPS = 1e-6
ISQRT2 = float(1.0 / np.sqrt(2.0))

GR = np.array([0, 1, 1, 1, 2, 2, 2, 3])          # blade -> grade (grade order)
GSLC = [(0, 1), (1, 3), (4, 3), (7, 1)]           # grade -> (blade start, count)


def _build_algebra():
    pc = lambda m: bin(m).count("1")
    order = sorted(range(NB), key=lambda m: (pc(m), m))
    cay = np.zeros((NB, NB, NB), np.float32)
    for a in range(NB):
        for b in range(NB):
            s, a_ = 0, a >> 1
            while a_:
                s += pc(a_ & b)
                a_ >>= 1
            cay[a, b, a ^ b] = -1.0 if s % 2 else 1.0
    cay = cay[np.ix_(order, order, order)]
    grades = np.array([pc(m) for m in order])
    paths = np.zeros((4, 4, 4), bool)
    ii, jj, kk = np.nonzero(cay)
    paths[grades[ii], grades[jj], grades[kk]] = True
    return cay, grades, np.argwhere(paths), order

_CAY, _GR, _PATHS, _ORDER = _build_algebra()
_POS = {m: p for p, m in enumerate(_ORDER)}
# IDX[j,k] = blade index i such that blade_i * blade_j contributes to blade_k
IDX = np.zeros((NB, NB), np.int64)
for _j in range(NB):
    for _k in range(NB):
        IDX[_j, _k] = _POS[_ORDER[_j] ^ _ORDER[_k]]


def _sigmoid(x):
    return 1.0 / (1.0 + np.exp(-x))


def _build_wtilde(gp_w, scale):
    """wt[j, k, c] = cayley_sign * w_full[c, g(i), g(j), g(k)] * scale, i=IDX[j,k]."""
    w_full = np.zeros((C, 4, 4, 4), np.float32)
    w_full[:, _PATHS[:, 0], _PATHS[:, 1], _PATHS[:, 2]] = gp_w
    wt = np.zeros((NB, NB, C), np.float32)
    for j in range(NB):
        for k in range(NB):
            i = IDX[j, k]
            wt[j, k] = _CAY[i, j, k] * w_full[:, _GR[i], _GR[j], _GR[k]] * scale
    return wt


def host_prep(inputs):
    """Build device-layout numpy arrays from the raw inputs."""
    f32 = lambda a: np.asarray(a, np.float32)
    # weights: [L, mat(0=lin,1=right,2=left), g, kb, mb, 128, 128] bf16 (lhsT chunks)
    wts = np.zeros((2, 3, 4, CB, CB, P, P), BF16)
    prm_cols = {}
    col = 0
    def alloc(name, n):
        nonlocal col
        prm_cols[name] = col
        col += n
    for l in range(2):
        alloc(f"blin{l}", 1); alloc(f"bleft{l}", 1)
        alloc(f"siluA{l}", 4); alloc(f"siluB{l}", 4)
        alloc(f"sa{l}", 4); alloc(f"tb{l}", 4)
        alloc(f"wt{l}", 64)
    alloc("lnA", 1)
    alloc("epsc", 1)
    prm = np.zeros((CB, P, col), np.float32)

    def put(name, arr):  # arr: [C] or [C, n]
        arr = np.asarray(arr, np.float32)
        if arr.ndim == 1:
            arr = arr[:, None]
        c0 = prm_cols[name]
        prm[:, :, c0:c0 + arr.shape[1]] = arr.reshape(CB, P, arr.shape[1])

    for l in range(2):
        w_lin = f32(inputs[f"w_lin{l}"]).copy()          # [O, I, 4]
        if l == 1:
            w_lin *= f32(inputs["ln_a0"])[None, :, None]  # fold prev-layer ln_a
        w_right = f32(inputs[f"w_right{l}"])
        w_left = f32(inputs[f"w_left{l}"]) * ISQRT2
        for g in range(4):
            for kb in range(CB):
                for mb in range(CB):
                    ks = slice(kb * P, (kb + 1) * P)
                    ms = slice(mb * P, (mb + 1) * P)
                    wts[l, 0, g, kb, mb] = w_lin[:, :, g].T[ks, ms].astype(BF16)
                    wts[l, 1, g, kb, mb] = w_right[:, :, g].T[ks, ms].astype(BF16)
                    wts[l, 2, g, kb, mb] = w_left[:, :, g].T[ks, ms].astype(BF16)
        put(f"blin{l}", f32(inputs[f"b_lin{l}"]))
        put(f"bleft{l}", f32(inputs[f"b_left{l}"]) * ISQRT2)
        put(f"siluA{l}", f32(inputs[f"silu_a{l}"]))
        put(f"siluB{l}", f32(inputs[f"silu_b{l}"]))
        sa = _sigmoid(f32(inputs[f"norm_a{l}"]))
        put(f"sa{l}", sa)
        put(f"tb{l}", 1.0 - sa + EPS)
        wt = _build_wtilde(f32(inputs[f"gp_w{l}"]), ISQRT2)  # [j, k, C]
        put(f"wt{l}", wt.reshape(64, C).T)
    put("lnA", f32(inputs["ln_a1"]))
    put("epsc", np.full(C, EPS, np.float32))

    ident = np.eye(P, dtype=BF16)
    ones = np.ones((P, 1), BF16)
    # x: [B, C, 8] -> [CB, 128, 8, B]
    xt = np.ascontiguousarray(
        f32(inputs["x"]).transpose(1, 2, 0).reshape(CB, P, NB, B_FULL))
    return xt, wts, prm, prm_cols, ident, ones


# ------------------------------------------------------------------ bass build
def build_program(prm_cols):
    import concourse.bass as bass
    import concourse.tile as tile
    from concourse import bacc, mybir

    fp32 = mybir.dt.float32
    bf16 = mybir.dt.bfloat16
    AF = mybir.ActivationFunctionType
    OP = mybir.AluOpType

    nc = bacc.Bacc("TRN2", target_bir_lowering=False, debug=False,
                   num_devices=N_CORES)

    x_d = nc.dram_tensor("x", [CB, P, NB, BC], fp32, kind="ExternalInput").ap()
    w_d = nc.dram_tensor("wts", [2, 3, 4, CB, CB, P, P], bf16,
                         kind="ExternalInput").ap()
    prm_d = nc.dram_tensor("prm", [CB, P, prm_cols["__total__"]], fp32,
                           kind="ExternalInput").ap()
    id_d = nc.dram_tensor("ident", [P, P], bf16, kind="ExternalInput").ap()
    ones_d = nc.dram_tensor("ones", [P, 1], bf16, kind="ExternalInput").ap()
    out_d = nc.dram_tensor("out", [CB, P, NB, BC], fp32,
                           kind="ExternalOutput").ap()

    with tile.TileContext(nc) as tc:
        _emit(tc, nc, bass, mybir, x_d, w_d, prm_d, id_d, ones_d, out_d,
              prm_cols, fp32, bf16, AF, OP)
    nc.compile()
    return nc


def _emit(tc, nc, bass, mybir, x_d, w_d, prm_d, id_d, ones_d, out_d,
          prm_cols, fp32, bf16, AF, OP):
    from contextlib import ExitStack
    ctx = ExitStack()
    consts = ctx.enter_context(tc.tile_pool(name="consts", bufs=1))
    # --- load constants ---------------------------------------------------
    wtiles = {}
    for l in range(2):
        for m in range(3):
            for g in range(4):
                for kb in range(CB):
                    for mb in range(CB):
                        t = consts.tile([P, P], bf16, name=f"w{l}{m}{g}{kb}{mb}",
                                        tag=f"w{l}{m}{g}{kb}{mb}")
                        nc.sync.dma_start(out=t, in_=w_d[l, m, g, kb, mb])
                        wtiles[(l, m, g, kb, mb)] = t
    ncols = prm_cols["__total__"]
    prm_t = consts.tile([P, CB, ncols], fp32, name="prm_t", tag="prm_t")
    for cb in range(CB):
        nc.sync.dma_start(out=prm_t[:, cb, :], in_=prm_d[cb])
    ident = consts.tile([P, P], bf16, name="ident_t", tag="ident_t")
    nc.sync.dma_start(out=ident, in_=id_d)
    ones = consts.tile([P, 1], bf16, name="ones_t", tag="ones_t")
    nc.sync.dma_start(out=ones, in_=ones_d)

    def pslice(name, cb, g0=0, n=1):
        c0 = prm_cols[name] + g0
        return prm_t[:, cb, c0:c0 + n]

    # --- pools ------------------------------------------------------------
    io = ctx.enter_context(tc.tile_pool(name="io", bufs=2))
    act = ctx.enter_context(tc.tile_pool(name="act", bufs=2))
    tmp = ctx.enter_context(tc.tile_pool(name="tmp", bufs=2))
    mmps = ctx.enter_context(tc.tile_pool(name="mmps", bufs=2, space="PSUM"))
    lgps = ctx.enter_context(tc.tile_pool(name="lgps", bufs=2, space="PSUM"))

    def mv_linear(src_bf, l, m):
        """per-grade matmuls of src [P, CB, NB, BT] bf16 -> bf16 tile
        [P, CB, NB, BT] (with blin bias for m==0)."""
        dst = act.tile([P, CB, NB, BT], bf16, name=f"h{l}{m}", tag=f"ev{m}",
                       bufs=1)
        for mb in range(CB):
            for kh in range(2):
                ps = mmps.tile([P, 4, BT], fp32, name=f"ps{l}{m}{mb}{kh}",
                               tag="mm")
                for kl in range(4):
                    k = kh * 4 + kl
                    g = int(GR[k])
                    for kb in range(CB):
                        nc.tensor.matmul(
                            ps[:, kl, :],
                            wtiles[(l, m, g, kb, mb)],
                            src_bf[:, kb, k, :],
                            start=(kb == 0),
                            stop=(kb == CB - 1),
                        )
                # evacuate PSUM -> SBUF bf16 (+ bias on scalar blade)
                if m == 0 and kh == 0:
                    nc.scalar.activation(
                        dst[:, mb, 0:1, :], ps[:, 0:1, :], AF.Identity,
                        bias=pslice(f"blin{l}", mb), scale=1.0)
                    nc.scalar.copy(dst[:, mb, 1:4, :], ps[:, 1:4, :])
                else:
                    nc.scalar.copy(dst[:, mb, kh * 4:kh * 4 + 4, :],
                                   ps[:, :, :])
        return dst

    def silu(h0, l):
        """h = sigmoid(a*inv+b)[grade] * h0  -> bf16 [P, CB, NB, BT]"""
        sq = tmp.tile([P, CB, 7, BT], bf16, name="sq", tag="sq", bufs=1)
        nc.scalar.square(sq, h0[:, :, 1:8, :])
        s1 = tmp.tile([P, CB, BT], bf16, name="s1", tag="s1", bufs=1)
        nc.vector.tensor_add(s1, sq[:, :, 0, :], sq[:, :, 1, :])
        inv1 = tmp.tile([P, CB, BT], bf16, name="inv1", tag="inv1", bufs=1)
        nc.vector.tensor_add(inv1, s1, sq[:, :, 2, :])
        s2 = tmp.tile([P, CB, BT], bf16, name="s2", tag="s2", bufs=1)
        nc.vector.tensor_add(s2, sq[:, :, 3, :], sq[:, :, 4, :])
        inv2 = tmp.tile([P, CB, BT], bf16, name="inv2", tag="inv2", bufs=1)
        nc.vector.tensor_add(inv2, s2, sq[:, :, 5, :])
        gate = tmp.tile([P, CB, 4, BT], bf16, name="gate", tag="gate", bufs=1)
        for cb in range(CB):
            ivs = [h0[:, cb, 0, :], inv1[:, cb, :], inv2[:, cb, :],
                   sq[:, cb, 6, :]]
            for g in range(4):
                nc.scalar.activation(
                    gate[:, cb, g, :], ivs[g], AF.Sigmoid,
                    bias=pslice(f"siluB{l}", cb, g),
                    scale=pslice(f"siluA{l}", cb, g))
        h = act.tile([P, CB, NB, BT], bf16, name="h", tag="h", bufs=2)
        for g in range(4):
            ks, nb = GSLC[g]
            nc.vector.tensor_mul(
                h[:, :, ks:ks + nb, :], h0[:, :, ks:ks + nb, :],
                gate[:, :, g:g + 1, :].broadcast_to([P, CB, nb, BT]))
        return h

    def grade_norm(yr, l):
        """xr = yr / (sa*|yr|_g + (1-sa+eps))  -> bf16 [P, CB, NB, BT]"""
        sqr = tmp.tile([P, CB, NB, BT], bf16, name="sqr", tag="sq", bufs=1)
        nc.scalar.square(sqr, yr)
        q1 = tmp.tile([P, CB, BT], bf16, name="q1", tag="s1", bufs=1)
        nc.vector.tensor_add(q1, sqr[:, :, 1, :], sqr[:, :, 2, :])
        q1b = tmp.tile([P, CB, BT], bf16, name="q1b", tag="inv1", bufs=1)
        nc.vector.tensor_add(q1b, q1, sqr[:, :, 3, :])
        q2 = tmp.tile([P, CB, BT], bf16, name="q2", tag="s2", bufs=1)
        nc.vector.tensor_add(q2, sqr[:, :, 4, :], sqr[:, :, 5, :])
        q2b = tmp.tile([P, CB, BT], bf16, name="q2b", tag="inv2", bufs=1)
        nc.vector.tensor_add(q2b, q2, sqr[:, :, 6, :])
        nrm = tmp.tile([P, CB, 4, BT], bf16, name="nrm", tag="gate", bufs=1)
        nc.scalar.activation(nrm[:, :, 0, :], sqr[:, :, 0, :], AF.Sqrt)
        nc.scalar.activation(nrm[:, :, 1, :], q1b, AF.Sqrt)
        nc.scalar.activation(nrm[:, :, 2, :], q2b, AF.Sqrt)
        nc.scalar.activation(nrm[:, :, 3, :], sqr[:, :, 7, :], AF.Sqrt)
        tp = tmp.tile([P, CB, 4, BT], fp32, name="tp", tag="tp", bufs=1)
        for cb in range(CB):
            for g in range(4):
                nc.scalar.activation(
                    tp[:, cb, g, :], nrm[:, cb, g, :], AF.Identity,
                    bias=pslice(f"tb{l}", cb, g),
                    scale=pslice(f"sa{l}", cb, g))
        uf = tmp.tile([P, CB, 4, BT], fp32, name="uf", tag="uf", bufs=1)
        nc.vector.reciprocal_approx_fast(
            uf.rearrange("p a b c -> p (a b c)"),
            tp.rearrange("p a b c -> p (a b c)"))
        ub = tmp.tile([P, CB, 4, BT], bf16, name="ub", tag="ub", bufs=1)
        nc.scalar.copy(ub, uf)
        xr = act.tile([P, CB, NB, BT], bf16, name="xr", tag="xr", bufs=1)
        for g in range(4):
            ks, nb = GSLC[g]
            nc.vector.tensor_mul(
                xr[:, :, ks:ks + nb, :], yr[:, :, ks:ks + nb, :],
                ub[:, :, g:g + 1, :].broadcast_to([P, CB, nb, BT]))
        return xr

    def left_geo_product(h, xr, l):
        """z = w_left-matmul(h) + bias + weighted geometric product, per
        contiguous (cb, kh) PSUM accumulation group."""
        z = act.tile([P, CB, NB, BT], bf16, name="z", tag="z", bufs=1)
        wcol = prm_cols[f"wt{l}"]
        for cb in range(CB):
            for kh in range(2):
                ps = lgps.tile([P, 4, BT], fp32, name=f"lg{l}{cb}{kh}",
                               tag="lg")
                for j in range(NB):
                    pjh = tmp.tile([P, 4, BT], bf16, name=f"pj{j}", tag="pj",
                                   bufs=3)
                    for kl in range(4):
                        k = kh * 4 + kl
                        i = int(IDX[j, k])
                        nc.vector.scalar_tensor_tensor(
                            out=pjh[:, kl, :],
                            in0=h[:, cb, i, :],
                            scalar=prm_t[:, cb, wcol + j * 8 + k:
                                         wcol + j * 8 + k + 1],
                            in1=xr[:, cb, j, :],
                            op0=OP.mult, op1=OP.mult)
                    for bh in range(2):
                        nc.tensor.matmul(
                            ps[:, bh * 2:bh * 2 + 2, :], ident,
                            pjh[:, bh * 2:bh * 2 + 2, :],
                            start=(j == 0), stop=False,
                            skip_group_check=True)
                for kl in range(4):
                    k = kh * 4 + kl
                    g = int(GR[k])
                    for kb in range(CB):
                        nc.tensor.matmul(
                            ps[:, kl, :],
                            wtiles[(l, 2, g, kb, cb)],
                            h[:, kb, k, :],
                            start=False, stop=(kb == CB - 1),
                            skip_group_check=True)
                if kh == 0:
                    nc.scalar.activation(
                        z[:, cb, 0:1, :], ps[:, 0:1, :], AF.Identity,
                        bias=pslice(f"bleft{l}", cb), scale=1.0)
                    nc.scalar.copy(z[:, cb, 1:4, :], ps[:, 1:4, :])
                else:
                    nc.scalar.copy(z[:, cb, 4:8, :], ps[:, :, :])
        return z

    def layer_norm(z, l, ci):
        """out = [ln_a *] z / (mean_c |z|_mv + eps); final layer writes DRAM."""
        sqz = tmp.tile([P, CB, NB, BT], bf16, name="sqz", tag="sq", bufs=1)
        nc.scalar.square(sqz, z)
        n2 = tmp.tile([P, CB, BT], fp32, name="n2", tag="n2", bufs=1)
        nc.vector.tensor_add(n2, sqz[:, :, 0, :], sqz[:, :, 1, :])
        for k in range(2, NB):
            nc.vector.tensor_add(n2, n2, sqz[:, :, k, :])
        nm = tmp.tile([P, CB, BT], bf16, name="nm", tag="nm", bufs=1)
        nc.scalar.activation(nm, n2, AF.Sqrt)
        mps = mmps.tile([1, BT], fp32, name="mps", tag="mm")
        for cb in range(CB):
            nc.tensor.matmul(mps, ones, nm[:, cb, :],
                             start=(cb == 0), stop=(cb == CB - 1))
        mi = tmp.tile([1, BT], fp32, name="mi", tag="mi", bufs=1)
        nc.scalar.activation(mi, mps, AF.Identity,
                             bias=prm_t[0:1, 0, prm_cols["epsc"]:
                                        prm_cols["epsc"] + 1],
                             scale=1.0 / C)
        rf = tmp.tile([1, BT], fp32, name="rf", tag="rf", bufs=1)
        nc.vector.reciprocal_approx_fast(rf, mi)
        rb = tmp.tile([1, BT], bf16, name="rb", tag="rb", bufs=1)
        nc.scalar.copy(rb, rf)
        rbc = tmp.tile([P, 1, 1, BT], bf16, name="rbc", tag="rbc", bufs=1)
        nc.gpsimd.partition_broadcast(rbc[:, 0, 0, :], rb)
        if l == 0:
            o = act.tile([P, CB, NB, BT], bf16, name="xnext", tag="xnext",
                         bufs=1)
            nc.vector.tensor_mul(
                o, z, rbc.broadcast_to([P, CB, NB, BT]))
            return o
        else:
            for cb in range(CB):
                o = io.tile([P, NB, BT], fp32, name="ostage", tag="ostage",
                            bufs=2)
                nc.vector.scalar_tensor_tensor(
                    out=o,
                    in0=z[:, cb, :, :],
                    scalar=pslice("lnA", cb),
                    in1=rbc[:, 0, :, :].broadcast_to([P, NB, BT]),
                    op0=OP.mult, op1=OP.mult)
                nc.sync.dma_start(out=out_d[cb, :, :, ci * BT:(ci + 1) * BT],
                                  in_=o)
            return None

    # --- main pipeline ----------------------------------------------------
    for ci in range(NCHUNK):
        xb = io.tile([P, CB, NB, BT], bf16, name="xb", tag="xb", bufs=2)
        for cb in range(CB):
            xf = io.tile([P, NB, BT], fp32, name="xf", tag="xf", bufs=2)
            nc.sync.dma_start(out=xf,
                              in_=x_d[cb, :, :, ci * BT:(ci + 1) * BT])
            nc.scalar.copy(xb[:, cb, :, :], xf)
        cur = xb
        for l in range(2):
            h0 = mv_linear(cur, l, 0)
            h = silu(h0, l)
            yr = mv_linear(h, l, 1)
            xr = grade_norm(yr, l)
            z = left_geo_product(h, xr, l)
            cur = layer_norm(z, l, ci)
    ctx.close()


# ------------------------------------------------------------------- frontend
_CACHE = {}
TRACE = False           # set True (e.g. from test.py) to capture exec_time_ns

def kernel(**inputs):
    from concourse.bass_utils import run_bass_kernel_spmd

    xt, wts, prm, prm_cols, ident, ones = host_prep(inputs)
    prm_cols = dict(prm_cols)
    prm_cols["__total__"] = prm.shape[2]

    if "nc" not in _CACHE:
        _CACHE["nc"] = build_program(prm_cols)
    nc = _CACHE["nc"]

    in_maps = []
    for core in range(N_CORES):
        sl = slice(core * BC, (core + 1) * BC)
        in_maps.append({
            "x": np.ascontiguousarray(xt[:, :, :, sl]),
            "wts": wts,
            "prm": prm,
            "ident": ident,
            "ones": ones,
        })
    res = run_bass_kernel_spmd(nc, in_maps, core_ids=list(range(N_CORES)),
                               trace=TRACE)
    _CACHE["exec_time_ns"] = res.exec_time_ns
    outs = [res.results[i]["out"] for i in range(N_CORES)]
    full = np.concatenate(outs, axis=3)          # [CB, P, NB, B]
    out = full.reshape(C, NB, B_FULL).transpose(2, 0, 1)
    return np.ascontiguousarray(out.astype(np.float32))
